# revision 18
# baseline (speedup 1.0000x reference)
"""BitLinear v11: phased o-halves. Weight image = [A: g0..31 x o(0:688)]
then [B: g0..31 x o(688:1376)]. Phase A's matmuls/drain/output overlap
phase B's weight stream; only B's tail remains exposed.
"""

import numpy as np

BATCH = 32
IN_F = 4096
OUT_F = 11008
GROUP = 128
N_GROUPS = IN_F // GROUP  # 32
N_CORES = 8
O_SHARD = OUT_F // N_CORES  # 1376
O_TILE = 344
O_HALF = 688
W_IMG_F = N_GROUPS * O_SHARD  # 44032
SLICE_GS = [6, 8, 8, 6, 3, 1]  # per phase
SCALE_NORM = 64.0

_nc_cache = []


def build_nc():
    import concourse.bacc as bacc
    import concourse.mybir as mybir
    import concourse.tile as tile

    f32 = mybir.dt.float32
    bf16 = mybir.dt.bfloat16
    fp8 = mybir.dt.float8e3

    nc = bacc.Bacc(None, target_bir_lowering=False)
    x_d = nc.dram_tensor("xT", [128, N_GROUPS * BATCH], bf16, kind="ExternalInput")
    w_d = nc.dram_tensor("wT", [128, W_IMG_F], fp8, kind="ExternalInput")
    y_d = nc.dram_tensor("y", [128, O_TILE], f32, kind="ExternalOutput")

    with tile.TileContext(nc) as tc:
        with tc.tile_pool(name="const", bufs=1) as const, tc.tile_pool(
            name="psum", bufs=1, space="PSUM"
        ) as psum:
            x_sb = const.tile([128, N_GROUPS, BATCH], bf16, tag="x_sb")
            # per phase h: [128, 32, 688]
            w_sb = const.tile([128, 2, N_GROUPS, O_HALF], fp8, tag="w_sb")
            y_sb = const.tile([128, O_TILE], f32, tag="y_sb")
            dummy_sb = const.tile([128, 2 * O_SHARD], fp8, tag="dummy_sb")

            nc.sync.dma_start(
                x_sb[:], x_d[:].rearrange("p (g b) -> p g b", g=N_GROUPS)
            )
            for h in range(2):
                base = h * N_GROUPS * O_HALF
                g0 = 0
                for gs in SLICE_GS:
                    nc.sync.dma_start(
                        w_sb[:, h, g0 : g0 + gs, :],
                        w_d[
                            :, base + g0 * O_HALF : base + (g0 + gs) * O_HALF
                        ].rearrange("p (g o) -> p g o", g=gs),
                    )
                    g0 += gs
            nc.sync.dma_start(dummy_sb[:], w_d[:, 0 : 2 * O_SHARD])

            ps = psum.tile([128, O_TILE], f32, tag="ps")
            for h in range(2):
                for g in range(N_GROUPS):
                    for jj in range(2):
                        j = 2 * h + jj
                        nc.tensor.matmul(
                            ps[32 * j : 32 * (j + 1), :],
                            x_sb[:, g, :],
                            w_sb[:, h, g, jj * O_TILE : (jj + 1) * O_TILE],
                            start=(g == 0),
                            stop=(g == N_GROUPS - 1),
                            tile_position=(0, 32 * j),
                        )
                # drain this phase's half [64 partitions, 344] as soon as its
                # chains stop; phase A's drain/output hide under B's stream
                nc.vector.tensor_copy(
                    y_sb[64 * h : 64 * (h + 1), :], ps[64 * h : 64 * (h + 1), :]
                )
                eng = nc.scalar if h == 0 else nc.sync
                eng.dma_start(
                    y_d[64 * h : 64 * (h + 1), :], y_sb[64 * h : 64 * (h + 1), :]
                )
            # tiny trailer after y so y itself doesn't pay the end-of-queue
            # crawl; the trailer's own completion is cheap (2KB)
            nc.sync.dma_start(dummy_sb[:, :16], w_d[:, 0:16])
    nc.finalize()
    return nc


def _pack_weights(signs_shard, scales_shard):
    import ml_dtypes

    w = signs_shard.astype(np.float32) * np.repeat(
        scales_shard.astype(np.float32) * SCALE_NORM, GROUP, axis=1
    )  # [O_SHARD, IN_F]
    # -> [g, p, o] with o split into halves: phase-major image
    t = w.T.reshape(N_GROUPS, GROUP, 2, O_HALF)  # [g, p, h, o]
    img = (
        t.transpose(1, 2, 0, 3)  # [p, h, g, o]
        .reshape(128, W_IMG_F)
        .astype(ml_dtypes.float8_e3m4)
    )
    return img


def _pack_x(x):
    import ml_dtypes

    return np.ascontiguousarray(
        (x.astype(np.float32).T / SCALE_NORM)
        .reshape(N_GROUPS, GROUP, BATCH)
        .transpose(1, 0, 2)
        .reshape(128, N_GROUPS * BATCH)
        .astype(ml_dtypes.bfloat16)
    )


def _shard_inputs(x, scales, signs):
    scales_r = np.asarray(scales).reshape(OUT_F, N_GROUPS)
    signs = np.asarray(signs)
    x_img = _pack_x(np.asarray(x))
    in_maps = []
    for c in range(N_CORES):
        lo, hi = c * O_SHARD, (c + 1) * O_SHARD
        in_maps.append(
            {"xT": x_img, "wT": _pack_weights(signs[lo:hi], scales_r[lo:hi])}
        )
    return in_maps


def _run(x, scales, signs, trace=False, tmpdir=None):
    from concourse import bass_utils

    if not _nc_cache:
        _nc_cache.append(build_nc())
    nc = _nc_cache[0]
    in_maps = _shard_inputs(x, scales, signs)
    res = bass_utils.run_bass_kernel_spmd(
        nc, in_maps, list(range(N_CORES)), trace=trace, tmpdir=tmpdir
    )
    parts = []
    for i in range(N_CORES):
        yc = np.asarray(res.results[i]["y"]).reshape(4, 32, O_TILE)
        parts.append(yc.transpose(1, 0, 2).reshape(BATCH, O_SHARD))
    out = np.concatenate(parts, axis=1)
    return np.ascontiguousarray(out).astype(np.float32), res


def kernel(x, scales, signs):
    out, _ = _run(x, scales, signs)
    return out
